# revision 7
# baseline (speedup 1.0000x reference)
"""Trainium2 Bass kernel for nn_MaskedAttention (B=2, N=2048, C=1024, H=16).

Sharding: batch x head-group over 8 cores (core c -> batch c//4, heads
4*(c%4)..4*(c%4)+3).  The reference's "faithful" head-scrambled reshape
means each head's output occupies a contiguous 128-row block of the
pre-projection matrix, so the output projection is row-parallel across
heads and needs no cross-core reduction.

Per-core pipeline (matmuls fp16 / bf16 / fp32r, all 1 cycle/row):
  1. QKV projection from fp16 x and weights: q,k stored transposed [d, n]
     with head pairs stacked on partitions (score matmuls are K=64 row
     tiles at base partitions 0/64, so the two heads of a pair run
     concurrently in disjoint row-groups); v stored [j, d] per head
     augmented with a ones column (denominator rides the AV matmul).
     QKV bias is applied on VectorE so ScalarE runs Exp exclusively.
  2. Scores sT[j, i] per 128x512 tile, causal tiles only, with the
     sub-diagonal column range trimmed from the matmul; exp via ScalarE
     (scale + constant -20 bias folded in); per-128-block causal mask via
     triangular multiply on diagonal tiles (no memsets - the trimmed
     region is never read).
  3. out^T = [V | 1]^T @ expT accumulated over j chunks, streaming only
     the causal column range; row 64 is the softmax denominator.
     Both heads' reciprocals are broadcast across partitions with a
     single K=2 selector matmul, and the normalize multiply scatters
     straight into the head-scrambled projection staging layout.
  4. Output projection + bias.
"""

import numpy as np

import concourse.bass as bass
import concourse.mybir as mybir
from concourse import tile
from concourse import library_config
from concourse.bass_utils import run_bass_kernel_spmd

B, N, C, H = 2, 2048, 1024, 16
D = C // H                 # 64
SCALE = D ** -0.5
EBIAS = -20.0
P = 128
NB = N // 512              # 4 i/n blocks
NJT = N // P               # 16 j tiles
F32 = mybir.dt.float32
F32R = mybir.dt.float32r
F16 = mybir.dt.float16
BF16 = mybir.dt.bfloat16
AF = mybir.ActivationFunctionType


def _emit(nc: bass.Bass, d: dict, repeats: int = 1):
    from contextlib import ExitStack

    with tile.TileContext(nc) as tc, ExitStack() as ctx:
        const = ctx.enter_context(tc.tile_pool(name="const", bufs=1))
        xT = const.tile([P, 8, N], F16)
        wqk = const.tile([P, 8, 512], F16)
        wv = const.tile([P, 8, 256], F16)
        bqk = const.tile([P, 4], F32)
        bv = const.tile([P, 256], F32)
        tri = const.tile([P, P], BF16)
        ebias = const.tile([P, 1], F32)
        qk = const.tile([P, 4, N], F32R)         # [p, {qq0,qq1,kk0,kk1}, n]
        vaug = const.tile([P, NJT, 4 * 65], BF16)
        wp = const.tile([P, 8, 1024], F16)
        bp = const.tile([P, 1024], F32)
        ones2 = const.tile([2, P], F32R)

        nc.sync.dma_start(bqk[:], d["b_qk"][:])
        nc.sync.dma_start(ones2[:], d["sel2"][:])
        nc.vector.memset(ebias[:], EBIAS)
        nc.vector.memset(vaug[:], 1.0)

        for _rep in range(repeats):
            # ---------------- QKV projection ----------------
            with tc.tile_pool(name="qkps", bufs=1, space="PSUM") as qkps, \
                 tc.tile_pool(name="vps", bufs=3, space="PSUM") as vps:
                for cc in range(8):
                    nc.sync.dma_start(xT[:, cc, :], d["xT"][cc])
                    nc.sync.dma_start(wqk[:, cc, :], d["w_qk"][cc])
                    nc.sync.dma_start(wv[:, cc, :], d["w_v"][cc])
                # deferred weight loads (needed later; keep xT DMAs first)
                nc.sync.dma_start(bv[:], d["b_v"][:])
                nc.sync.dma_start(tri[:], d["tri"][:])
                nc.sync.dma_start(wp[:], d["w_p"].rearrange("k p m -> p k m"))
                nc.sync.dma_start(bp[:], d["b_p"][:])

                # mb-outer / nb-inner keeps the stationary weight loaded
                # across 4 matmuls; (0,2,1,3) readies pair 0's q,k first
                for mb in (0, 2, 1, 3):
                    pss = [qkps.tile([P, 512], F32, tag=f"qk{nb}",
                                     name=f"qkps{nb}") for nb in range(NB)]
                    for cc in range(8):
                        for nb in range(NB):
                            nc.tensor.matmul(
                                pss[nb][:],
                                wqk[:, cc, P * mb:P * mb + P],
                                xT[:, cc, 512 * nb:512 * nb + 512],
                                start=(cc == 0), stop=(cc == 7),
                            )
                    with nc.allow_low_precision(reason="f32r qk store"):
                        for nb in range(NB):
                            nc.vector.tensor_scalar_add(
                                qk[:, mb, 512 * nb:512 * nb + 512], pss[nb][:],
                                bqk[:, mb:mb + 1],
                            )
                for jt in range(NJT):
                    ps = vps.tile([P, 256], F32, tag="v")
                    for cc in range(8):
                        nc.tensor.matmul(
                            ps[:],
                            xT[:, cc, P * jt:P * jt + P],
                            wv[:, cc, :],
                            start=(cc == 0), stop=(cc == 7),
                        )
                    vview = vaug[:, jt, :].rearrange("p (h x) -> p h x", x=65)[:, :, 0:64]
                    nc.vector.tensor_add(
                        out=vview,
                        in0=ps[:].rearrange("p (h x) -> p h x", x=64),
                        in1=bv[:].rearrange("p (h x) -> p h x", x=64),
                    )

            # ---------------- attention + projection ----------------
            with tc.tile_pool(name="att", bufs=1) as att, \
                 tc.tile_pool(name="post", bufs=2) as post, \
                 tc.tile_pool(name="sps", bufs=2, space="PSUM") as sps, \
                 tc.tile_pool(name="avps", bufs=1, space="PSUM") as avps, \
                 tc.tile_pool(name="bcps", bufs=1, space="PSUM") as bcps, \
                 tc.tile_pool(name="pps", bufs=2, space="PSUM") as pps:
                for pair in range(2):
                    stages = [post.tile([P, 8, P], F16, tag=f"stage{hp}",
                                        name=f"stage{hp}") for hp in range(2)]
                    for m in range(NB):
                        njt = 4 * (m + 1)
                        expTs = [att.tile([P, NJT, 512], BF16, tag=f"expT{hp}",
                                          name=f"expT{hp}")
                                 for hp in range(2)]
                        for jt in range(njt):
                            t = jt - 4 * m
                            lo = P * t if t > 0 else 0
                            pss = []
                            for hp in range(2):
                                row = 64 * hp
                                ps_s = sps.tile([P, 512], F32, tag="sT")
                                nc.tensor.matmul(
                                    ps_s[:, lo:512],
                                    qk[row:row + 64, 2 + pair, P * jt:P * jt + P],
                                    qk[row:row + 64, pair,
                                       512 * m + lo:512 * m + 512],
                                    start=True, stop=True,
                                )
                                pss.append(ps_s)
                            for hp in range(2):
                                expT, ps_s = expTs[hp], pss[hp]
                                nc.scalar.activation(
                                    expT[:, jt, lo:512], ps_s[:, lo:512],
                                    AF.Exp, bias=ebias[:], scale=SCALE)
                                if t >= 0:
                                    nc.vector.tensor_mul(
                                        out=expT[:, jt, lo:lo + P],
                                        in0=expT[:, jt, lo:lo + P], in1=tri[:])
                        ps_os = []
                        for hp in range(2):
                            h = 2 * pair + hp
                            expT = expTs[hp]
                            ps_o = avps.tile([65, 512], F32, tag=f"av{hp}")
                            for jt in range(njt):
                                t = jt - 4 * m
                                lo = P * t if t > 0 else 0
                                nc.tensor.matmul(
                                    ps_o[:, lo:512],
                                    vaug[:, jt, 65 * h:65 * h + 65],
                                    expT[:, jt, lo:512],
                                    start=(jt == 0), stop=(jt == njt - 1),
                                    skip_group_check=True,
                                )
                            ps_os.append(ps_o)
                        bcp = bcps.tile([64, 2, 512], F32, tag="bc")
                        with nc.allow_low_precision(
                                reason="f32r tag for fast broadcast matmul"):
                            for hp in range(2):
                                rec = post.tile([1, 512], F32R, tag=f"rec{hp}")
                                nc.vector.reciprocal(
                                    rec[:], ps_os[hp][64:65, :])
                                nc.tensor.matmul(bcp[:, hp, :],
                                                 ones2[0:1, 0:64], rec[:],
                                                 start=True, stop=True)
                        bc = post.tile([64, 2, 512], F32, tag="bc")
                        nc.vector.tensor_copy(bc[:], bcp[:])
                        # normalize + scatter into head-scrambled staging:
                        # stage[d + 64e, k, 32m + q] = out[i, d] / den(i)
                        # for token i = 512m + 16q + 2k + e.
                        for hp in range(2):
                            pv = ps_os[hp][0:64, :].rearrange(
                                "p (q k e) -> p k q e", k=8, e=2)
                            bv2 = bc[:, hp, :].rearrange(
                                "p (q k e) -> p k q e", k=8, e=2)
                            for e in range(2):
                                nc.vector.tensor_mul(
                                    out=stages[hp][64 * e:64 * e + 64, :,
                                                   32 * m:32 * m + 32],
                                    in0=pv[:, :, :, e:e + 1],
                                    in1=bv2[:, :, :, e:e + 1])
                    # projection per head
                    for hp in range(2):
                        h = 2 * pair + hp
                        for mb2 in range(2):
                            psp = pps.tile([P, 512], F32, tag="proj")
                            for k in range(8):
                                nc.tensor.matmul(
                                    psp[:],
                                    stages[hp][:, k, :],
                                    wp[:, k, 512 * mb2:512 * mb2 + 512],
                                    start=(k == 0), stop=(k == 7),
                                )
                            osb = post.tile([P, 512], F32, tag="osb")
                            nc.vector.tensor_add(
                                out=osb[:], in0=psp[:],
                                in1=bp[:, 512 * mb2:512 * mb2 + 512])
                            nc.sync.dma_start(
                                d["out"][P * h:P * h + P, 512 * mb2:512 * mb2 + 512],
                                osb[:])



def _fix_bir_for_walrus(bir: bytes) -> bytes:
    """Split multi-semaphore-wait instructions for walrus builds that
    support only one sync-wait command per instruction: extra waits are
    hoisted onto same-engine NoOps inserted immediately before.  ISA-class
    (custom Pool) instructions get ALL waits hoisted."""
    import json as _json

    d = _json.loads(bir)
    uid = [0]
    for fn in d["functions"]:
        for blk in fn["blocks"]:
            out = []
            for inst in blk["instructions"]:
                si = inst.get("sync_info")
                waits = (si or {}).get("on_wait") or []
                keep = 0 if "isa_opcode" in inst else 1
                if len(waits) > keep:
                    hoist, rest = waits[:len(waits) - keep], waits[len(waits) - keep:]
                    for w in hoist:
                        uid[0] += 1
                        out.append({
                            "name": f"I-wsplit-{uid[0]}",
                            "opcode": "NoOp",
                            "engine": inst["engine"],
                            "ins": [],
                            "outs": [],
                            "sync_info": {"on_wait": [w], "on_update": []},
                        })
                    si["on_wait"] = rest
                out.append(inst)
            blk["instructions"] = out
    return _json.dumps(d).encode()


_NC_CACHE = None


def build_bass(repeats: int = 1) -> bass.Bass:
    global _NC_CACHE
    if repeats == 1 and _NC_CACHE is not None:
        return _NC_CACHE
    nc = bass.Bass("TRN2", target_bir_lowering=False, debug=False,
                   enable_asserts=False, num_devices=8)
    d = {
        "xT": nc.dram_tensor("xT", [8, P, N], F16, kind="ExternalInput").ap(),
        "w_qk": nc.dram_tensor("w_qk", [8, P, 512], F16, kind="ExternalInput").ap(),
        "w_v": nc.dram_tensor("w_v", [8, P, 256], F16, kind="ExternalInput").ap(),
        "b_qk": nc.dram_tensor("b_qk", [P, 4], F32, kind="ExternalInput").ap(),
        "b_v": nc.dram_tensor("b_v", [P, 256], F32, kind="ExternalInput").ap(),
        "w_p": nc.dram_tensor("w_p", [8, P, 1024], F16, kind="ExternalInput").ap(),
        "b_p": nc.dram_tensor("b_p", [P, 1024], F32, kind="ExternalInput").ap(),
        "tri": nc.dram_tensor("tri", [P, P], BF16, kind="ExternalInput").ap(),
        "sel2": nc.dram_tensor("sel2", [2, P], F32R, kind="ExternalInput").ap(),
        "out": nc.dram_tensor("out", [512, 1024], F32, kind="ExternalOutput").ap(),
    }
    _emit(nc, d, repeats=repeats)
    _orig_to_json = nc.to_json_bytes
    nc.to_json_bytes = lambda: _fix_bir_for_walrus(_orig_to_json())
    if repeats == 1:
        _NC_CACHE = nc
    return nc


def _core_inputs(core: int, x, w_qkv, b_qkv, w_proj, b_proj) -> dict:
    import ml_dtypes

    b = core // 4
    h0 = 4 * (core % 4)
    xT = np.ascontiguousarray(x[b].T.reshape(8, P, N), np.float16)

    rows, brows = [], []
    for sec in (0, 1):                       # q section then k section
        for p in range(2):
            for e in range(2):
                h = h0 + 2 * p + e
                rows.append(w_qkv[sec * C + D * h: sec * C + D * h + D])
                brows.append(b_qkv[sec * C + D * h: sec * C + D * h + D])
    W_stack = np.concatenate(rows, 0)        # [512, 1024]
    w_qk = np.ascontiguousarray(W_stack.T.reshape(8, P, 512), np.float16)
    b_qk = np.ascontiguousarray(
        np.concatenate(brows, 0).reshape(4, P).T, np.float32)

    W_v4 = w_qkv[2 * C + D * h0: 2 * C + D * h0 + 256]
    w_v = np.ascontiguousarray(W_v4.T.reshape(8, P, 256), np.float16)
    b_v = np.ascontiguousarray(
        np.broadcast_to(b_qkv[2 * C + D * h0: 2 * C + D * h0 + 256], (P, 256)),
        np.float32)

    w_p = np.ascontiguousarray(w_proj.T.reshape(8, P, 1024), np.float16)
    b_p = np.ascontiguousarray(np.broadcast_to(b_proj, (P, 1024)), np.float32)
    tri = (np.arange(P)[None, :] >= np.arange(P)[:, None]).astype(ml_dtypes.bfloat16)
    sel2 = np.zeros((2, P), np.float32)
    sel2[0, 0:64] = 1.0
    sel2[1, 64:P] = 1.0
    return {"xT": xT, "w_qk": w_qk, "w_v": w_v, "b_qk": b_qk, "b_v": b_v,
            "w_p": w_p, "b_p": b_p, "tri": tri, "sel2": sel2}


def _is_causal(mask: np.ndarray) -> bool:
    if mask.shape != (B, N, N):
        return False
    tril = np.tril(np.ones((N, N), bool))
    return bool(all(np.array_equal(mask[i], tril) for i in range(mask.shape[0])))


def _numpy_fallback(x, attention_mask, w_qkv, b_qkv, w_proj, b_proj):
    b, n, c = x.shape
    qkv = x @ w_qkv.T + b_qkv
    qkv = qkv.reshape(b, n, 3, H, D).transpose(2, 0, 3, 1, 4)
    q, k, v = qkv[0], qkv[1], qkv[2]
    dots = np.einsum("bhid,bhjd->bhij", q, k) * SCALE
    mask_value = -np.finfo(dots.dtype).max
    dots = np.where(attention_mask[:, None, :, :], dots, mask_value)
    dots = dots - dots.max(axis=-1, keepdims=True)
    e = np.exp(dots)
    attn = e / e.sum(axis=-1, keepdims=True)
    out = np.einsum("bhij,bhjd->bhid", attn, v)
    out = out.reshape(b, n, c)
    return (out @ w_proj.T + b_proj).astype(np.float32)


def kernel(**inputs) -> np.ndarray:
    x = np.asarray(inputs["x"], np.float32)
    mask = np.asarray(inputs["attention_mask"])
    w_qkv = np.asarray(inputs["w_qkv"], np.float32)
    b_qkv = np.asarray(inputs["b_qkv"], np.float32)
    w_proj = np.asarray(inputs["w_proj"], np.float32)
    b_proj = np.asarray(inputs["b_proj"], np.float32)

    if not _is_causal(mask):
        return _numpy_fallback(x, mask, w_qkv, b_qkv, w_proj, b_proj)

    nc = build_bass()
    in_maps = [_core_inputs(c, x, w_qkv, b_qkv, w_proj, b_proj)
               for c in range(8)]
    res = run_bass_kernel_spmd(nc, in_maps, core_ids=list(range(8)))
    out = np.empty((B, N, C), np.float32)
    for c in range(8):
        b = c // 4
        h0 = 4 * (c % 4)
        out[b, P * h0:P * h0 + 512, :] = res.results[c]["out"]
    return out


# revision 35
# speedup vs baseline: 2.1094x; 2.1094x over previous
"""Trainium2 Bass kernel for nn_MaskedAttention (B=2, N=2048, C=1024, H=16).

Sharding: batch x head-group over 8 cores (core c -> batch c//4, heads
4*(c%4)..4*(c%4)+3).  The reference's "faithful" head-scrambled reshape
means each head's output occupies a contiguous 128-row block of the
pre-projection matrix, so the output projection is row-parallel across
heads and needs no cross-core reduction.

Per-core pipeline (matmuls fp16 / bf16 / fp32r, all 1 cycle/row):
  1. QKV projection from fp16 x and weights: q,k stored transposed [d, n]
     with head pairs stacked on partitions (score matmuls are K=64 row
     tiles at base partitions 0/64, so the two heads of a pair run
     concurrently in disjoint row-groups); v stored [j, d] per head
     augmented with a ones column (denominator rides the AV matmul).
     QKV bias is applied on VectorE so ScalarE runs Exp exclusively.
  2. Scores sT[j, i] per 128x512 tile, causal tiles only, with the
     sub-diagonal column range trimmed from the matmul; exp via ScalarE
     (scale + constant -20 bias folded in); per-128-block causal mask via
     triangular multiply on diagonal tiles (no memsets - the trimmed
     region is never read).
  3. out^T = [V | 1]^T @ expT accumulated over j chunks, streaming only
     the causal column range; row 64 is the softmax denominator.
     Both heads' reciprocals are broadcast across partitions with a
     single K=2 selector matmul, and the normalize multiply scatters
     straight into the head-scrambled projection staging layout.
  4. Output projection + bias.
"""

import numpy as np

import concourse.bass as bass
import concourse.mybir as mybir
from concourse import tile
from concourse import library_config
from concourse.bass_utils import run_bass_kernel_spmd

B, N, C, H = 2, 2048, 1024, 16
D = C // H                 # 64
SCALE = D ** -0.5
EBIAS = -20.0
P = 128
NB = N // 512              # 4 i/n blocks
NJT = N // P               # 16 j tiles
F32 = mybir.dt.float32
F32R = mybir.dt.float32r
F16 = mybir.dt.float16
BF16 = mybir.dt.bfloat16
AF = mybir.ActivationFunctionType


def _emit(nc: bass.Bass, d: dict, repeats: int = 1):
    from contextlib import ExitStack

    with tile.TileContext(nc) as tc, ExitStack() as ctx:
        const = ctx.enter_context(tc.tile_pool(name="const", bufs=1))
        xT = const.tile([P, 8, N], F16)
        wqk = const.tile([P, 8, 512], F16)
        wv = const.tile([P, 8, 256], F16)
        bqk = const.tile([P, 4], F32)
        bv = const.tile([P, 256], F32)
        tri = const.tile([P, P], BF16)
        ebias = const.tile([P, 1], F32)
        qk = const.tile([P, 4, N], F32R)         # [p, {qq0,qq1,kk0,kk1}, n]
        vaug = const.tile([P, NJT, 4 * 65], BF16)
        wp = const.tile([P, 8, 1024], F16)
        bp = const.tile([P, 1024], F32)
        ones2 = const.tile([2, P], F32R)

        nc.sync.dma_start(bqk[:], d["b_qk"][:])
        nc.sync.dma_start(ones2[:], d["sel2"][:])
        nc.vector.memset(ebias[:], EBIAS)
        nc.vector.memset(vaug[:], 1.0)

        from collections import deque

        for _rep in range(repeats):
            # One psum ring (pps, 2x [P,512] banks) is shared by QKV
            # accumulation, the reciprocal broadcast, and the projection;
            # each user allocates and finishes a slot atomically so ring
            # order can never deadlock against PE program order.
            with tc.tile_pool(name="att2", bufs=2) as att2, \
                 tc.tile_pool(name="post", bufs=2) as post, \
                 tc.tile_pool(name="sps", bufs=2, space="PSUM") as sps, \
                 tc.tile_pool(name="avps", bufs=1, space="PSUM") as avps, \
                 tc.tile_pool(name="pps", bufs=2, space="PSUM") as pps:
                # wqk first (every qk block needs all of it), then xT in
                # column waves so qk_block(mb, nb) unblocks after wave nb.
                # Even/odd chunks go down separate DGE queues (SP + idle
                # GpSimd) so transfers overlap instead of serializing.
                qs = (nc.sync, nc.gpsimd)
                halves = ((0, 4), (4, 8))
                xTd = d["xT"].rearrange("c p m -> p c m")
                wqkd = d["w_qk"].rearrange("c p m -> p c m")
                for q, (c0, c1) in zip(qs, halves):
                    q.dma_start(wqk[:, c0:c1, :], wqkd[:, c0:c1, :])
                    q.dma_start(xT[:, c0:c1, 0:512], xTd[:, c0:c1, 0:512])
                for nb in range(1, NB):
                    for q, (c0, c1) in zip(qs, halves):
                        q.dma_start(
                            xT[:, c0:c1, 512 * nb:512 * nb + 512],
                            xTd[:, c0:c1, 512 * nb:512 * nb + 512])
                    if nb == 1:
                        for q, (c0, c1) in zip(qs, halves):
                            q.dma_start(wv[:, c0:c1, :],
                                        d["w_v"].rearrange("c p m -> p c m")
                                        [:, c0:c1, :])
                        nc.gpsimd.dma_start(bv[:], d["b_v"][:])
                        nc.gpsimd.dma_start(tri[:], d["tri"][:])
                nc.gpsimd.dma_start(wp[:], d["w_p"].rearrange("k p m -> p k m"))
                nc.gpsimd.dma_start(bp[:], d["b_p"][:])

                def qk_block(mb, nb):
                    ps = pps.tile([P, 512], F32, tag="proj", name="qkps")
                    for cc in range(8):
                        nc.tensor.matmul(
                            ps[:],
                            wqk[:, cc, P * mb:P * mb + P],
                            xT[:, cc, 512 * nb:512 * nb + 512],
                            start=(cc == 0), stop=(cc == 7),
                        )
                    with nc.allow_low_precision(reason="f32r qk store"):
                        nc.vector.tensor_scalar_add(
                            qk[:, mb, 512 * nb:512 * nb + 512], ps[:],
                            bqk[:, mb:mb + 1],
                        )

                def v_block(jt):
                    ps = pps.tile([P, 512], F32, tag="proj", name="vps")
                    for cc in range(8):
                        nc.tensor.matmul(
                            ps[:, 0:256],
                            xT[:, cc, P * jt:P * jt + P],
                            wv[:, cc, :],
                            start=(cc == 0), stop=(cc == 7),
                        )
                    vview = vaug[:, jt, :].rearrange(
                        "p (h x) -> p h x", x=65)[:, :, 0:64]
                    nc.vector.tensor_add(
                        out=vview,
                        in0=ps[:, 0:256].rearrange("p (h x) -> p h x", x=64),
                        in1=bv[:].rearrange("p (h x) -> p h x", x=64),
                    )

                def proj_block(pair, hp, mb2, stage):
                    h = 2 * pair + hp
                    psp = pps.tile([P, 512], F32, tag="proj")
                    for k in range(8):
                        nc.tensor.matmul(
                            psp[:],
                            stage[:, k, :],
                            wp[:, k, 512 * mb2:512 * mb2 + 512],
                            start=(k == 0), stop=(k == 7),
                        )
                    osb = post.tile([P, 512], F32, tag="osb")
                    nc.vector.tensor_add(
                        out=osb[:], in0=psp[:],
                        in1=bp[:, 512 * mb2:512 * mb2 + 512])
                    nc.sync.dma_start(
                        d["out"][P * h:P * h + P, 512 * mb2:512 * mb2 + 512],
                        osb[:])

                # upfront: pair 0's q,k for the first two column waves and
                # v for j-tiles 0-7; the rest of QKV is woven into
                # attention stall windows in dependency-safe order
                filler = deque()
                pending = set()

                def push(key, f, args):
                    filler.append((key, f, args))
                    pending.add(key)

                for nb in range(2):
                    qk_block(0, nb)
                    qk_block(2, nb)
                    for jt in range(4 * nb, 4 * nb + 4):
                        v_block(jt)
                for nb in range(2, NB):
                    push(("qk", 0, nb), qk_block, (0, nb))
                    push(("qk", 2, nb), qk_block, (2, nb))
                for jt in range(8, NJT):
                    push(("v", jt), v_block, (jt,))
                for mb in (1, 3):
                    for nb in range(NB):
                        push(("qk", mb, nb), qk_block, (mb, nb))

                def pop_filler():
                    if filler:
                        key, f, args = filler.popleft()
                        pending.discard(key)
                        f(*args)

                def need(keys):
                    # force-emit queued blocks a later instruction depends
                    # on (PE is in-order: emitting a consumer before its
                    # producer would deadlock the engine queue)
                    while pending & keys:
                        pop_filler()

                def attention_pair(pair, stages, late=(), tail_proj=False):
                    for m in range(NB):
                        njt = 4 * (m + 1)
                        if late and m == NB - 1:
                            for key, f, args in late:
                                push(key, f, args)
                        need({("qk", pair, m)} |
                             {("qk", 2 + pair, nb2) for nb2 in range(m + 1)} |
                             {("v", jt) for jt in range(njt)})
                        # bufs=2: exp for block m+1 may proceed while AV of
                        # block m is still reading the previous tiles
                        expT = att2.tile([P, NJT, 2, 512], BF16, tag="expT",
                                         name="expT")
                        for jt in range(njt):
                            t = jt - 4 * m
                            lo = P * t if t > 0 else 0
                            # both heads of the pair into one 2-bank psum
                            # tile so a single exp covers 2x512 columns
                            ps_s = sps.tile([P, 2, 512], F32, tag="sT")
                            for hp in range(2):
                                row = 64 * hp
                                nc.tensor.matmul(
                                    ps_s[:, hp, lo:512],
                                    qk[row:row + 64, 2 + pair, P * jt:P * jt + P],
                                    qk[row:row + 64, pair,
                                       512 * m + lo:512 * m + 512],
                                    start=True, stop=True,
                                )
                            nc.scalar.activation(
                                expT[:, jt, :, lo:512], ps_s[:, :, lo:512],
                                AF.Exp, bias=ebias[:], scale=SCALE)
                            if t >= 0:
                                for hp in range(2):
                                    nc.vector.tensor_mul(
                                        out=expT[:, jt, hp, lo:lo + P],
                                        in0=expT[:, jt, hp, lo:lo + P],
                                        in1=tri[:])
                            if jt % 2 == 1:
                                pop_filler()
                        ps_os, bcs = [], []
                        for hp in range(2):
                            h = 2 * pair + hp
                            ps_o = avps.tile([P, 512], F32, tag=f"av{hp}")
                            for jt in range(njt):
                                t = jt - 4 * m
                                lo = P * t if t > 0 else 0
                                nc.tensor.matmul(
                                    ps_o[0:65, lo:512],
                                    vaug[:, jt, 65 * h:65 * h + 65],
                                    expT[:, jt, hp, lo:512],
                                    start=(jt == 0), stop=(jt == njt - 1),
                                    skip_group_check=True,
                                )
                            ps_os.append(ps_o)
                            # broadcast 1/den across partitions via a K=1
                            # ones matmul riding the shared psum ring
                            with nc.allow_low_precision(
                                    reason="f32r tag for fast broadcast matmul"):
                                rec = post.tile([1, 512], F32R, tag=f"rec{hp}")
                                nc.vector.reciprocal(rec[:], ps_o[64:65, :])
                                bcp = pps.tile([P, 512], F32, tag="proj",
                                               name="bcp")
                                nc.tensor.matmul(bcp[0:64, :],
                                                 ones2[0:1, 0:64], rec[:],
                                                 start=True, stop=True)
                            bc = post.tile([64, 512], F32, tag=f"bc{hp}")
                            nc.vector.tensor_copy(bc[:], bcp[0:64, :])
                            bcs.append(bc)
                            pop_filler()
                        # normalize + scatter into head-scrambled staging:
                        # stage[d + 64e, k, 32m + q] = out[i, d] / den(i)
                        # for token i = 512m + 16q + 2k + e.  (DVE reads at
                        # most one PSUM operand, so the broadcast goes
                        # through SBUF.)
                        for hp in range(2):
                            pv = ps_os[hp][0:64, :].rearrange(
                                "p (q k e) -> p k q e", k=8, e=2)
                            bv2 = bcs[hp][:].rearrange(
                                "p (q k e) -> p k q e", k=8, e=2)
                            for e in range(2):
                                nc.vector.tensor_mul(
                                    out=stages[hp][64 * e:64 * e + 64, :,
                                                   32 * m:32 * m + 32],
                                    in0=pv[:, :, :, e:e + 1],
                                    in1=bv2[:, :, :, e:e + 1])
                            # last block: this head's projection can go as
                            # soon as its own staging is complete
                            if tail_proj and m == NB - 1:
                                for mb2 in range(2):
                                    proj_block(pair, hp, mb2, stages[hp])
                        pop_filler()

                stages0 = [post.tile([P, 8, P], F16, tag=f"stage{hp}",
                                     name=f"stage{hp}") for hp in range(2)]
                attention_pair(0, stages0)
                # pair 0's projection fills pair 1's final-block windows
                late = [(("proj", hp, mb2), proj_block,
                         (0, hp, mb2, stages0[hp]))
                        for hp in range(2) for mb2 in range(2)]
                stages1 = [post.tile([P, 8, P], F16, tag=f"stage{hp}",
                                     name=f"stage{hp}") for hp in range(2)]
                attention_pair(1, stages1, late=late, tail_proj=True)
                while filler:
                    pop_filler()



def _fix_bir_for_walrus(bir: bytes) -> bytes:
    """Split multi-semaphore-wait instructions for walrus builds that
    support only one sync-wait command per instruction: extra waits are
    hoisted onto same-engine NoOps inserted immediately before.  ISA-class
    (custom Pool) instructions get ALL waits hoisted."""
    import json as _json

    d = _json.loads(bir)
    uid = [0]
    for fn in d["functions"]:
        for blk in fn["blocks"]:
            out = []
            for inst in blk["instructions"]:
                si = inst.get("sync_info")
                waits = (si or {}).get("on_wait") or []
                keep = 0 if "isa_opcode" in inst else 1
                if len(waits) > keep:
                    hoist, rest = waits[:len(waits) - keep], waits[len(waits) - keep:]
                    for w in hoist:
                        uid[0] += 1
                        out.append({
                            "name": f"I-wsplit-{uid[0]}",
                            "opcode": "NoOp",
                            "engine": inst["engine"],
                            "ins": [],
                            "outs": [],
                            "sync_info": {"on_wait": [w], "on_update": []},
                        })
                    si["on_wait"] = rest
                out.append(inst)
            blk["instructions"] = out
    return _json.dumps(d).encode()


_NC_CACHE = None


def build_bass(repeats: int = 1) -> bass.Bass:
    global _NC_CACHE
    if repeats == 1 and _NC_CACHE is not None:
        return _NC_CACHE
    nc = bass.Bass("TRN2", target_bir_lowering=False, debug=False,
                   enable_asserts=False, num_devices=8)
    d = {
        "xT": nc.dram_tensor("xT", [8, P, N], F16, kind="ExternalInput").ap(),
        "w_qk": nc.dram_tensor("w_qk", [8, P, 512], F16, kind="ExternalInput").ap(),
        "w_v": nc.dram_tensor("w_v", [8, P, 256], F16, kind="ExternalInput").ap(),
        "b_qk": nc.dram_tensor("b_qk", [P, 4], F32, kind="ExternalInput").ap(),
        "b_v": nc.dram_tensor("b_v", [P, 256], F32, kind="ExternalInput").ap(),
        "w_p": nc.dram_tensor("w_p", [8, P, 1024], F16, kind="ExternalInput").ap(),
        "b_p": nc.dram_tensor("b_p", [P, 1024], F32, kind="ExternalInput").ap(),
        "tri": nc.dram_tensor("tri", [P, P], BF16, kind="ExternalInput").ap(),
        "sel2": nc.dram_tensor("sel2", [2, P], F32R, kind="ExternalInput").ap(),
        "out": nc.dram_tensor("out", [512, 1024], F32, kind="ExternalOutput").ap(),
    }
    _emit(nc, d, repeats=repeats)
    _orig_to_json = nc.to_json_bytes
    nc.to_json_bytes = lambda: _fix_bir_for_walrus(_orig_to_json())
    if repeats == 1:
        _NC_CACHE = nc
    return nc


def _core_inputs(core: int, x, w_qkv, b_qkv, w_proj, b_proj) -> dict:
    import ml_dtypes

    b = core // 4
    h0 = 4 * (core % 4)
    xT = np.ascontiguousarray(x[b].T.reshape(8, P, N), np.float16)

    rows, brows = [], []
    for sec in (0, 1):                       # q section then k section
        for p in range(2):
            for e in range(2):
                h = h0 + 2 * p + e
                rows.append(w_qkv[sec * C + D * h: sec * C + D * h + D])
                brows.append(b_qkv[sec * C + D * h: sec * C + D * h + D])
    W_stack = np.concatenate(rows, 0)        # [512, 1024]
    w_qk = np.ascontiguousarray(W_stack.T.reshape(8, P, 512), np.float16)
    b_qk = np.ascontiguousarray(
        np.concatenate(brows, 0).reshape(4, P).T, np.float32)

    W_v4 = w_qkv[2 * C + D * h0: 2 * C + D * h0 + 256]
    w_v = np.ascontiguousarray(W_v4.T.reshape(8, P, 256), np.float16)
    b_v = np.ascontiguousarray(
        np.broadcast_to(b_qkv[2 * C + D * h0: 2 * C + D * h0 + 256], (P, 256)),
        np.float32)

    w_p = np.ascontiguousarray(w_proj.T.reshape(8, P, 1024), np.float16)
    b_p = np.ascontiguousarray(np.broadcast_to(b_proj, (P, 1024)), np.float32)
    tri = (np.arange(P)[None, :] >= np.arange(P)[:, None]).astype(ml_dtypes.bfloat16)
    sel2 = np.zeros((2, P), np.float32)
    sel2[0, 0:64] = 1.0
    sel2[1, 64:P] = 1.0
    return {"xT": xT, "w_qk": w_qk, "w_v": w_v, "b_qk": b_qk, "b_v": b_v,
            "w_p": w_p, "b_p": b_p, "tri": tri, "sel2": sel2}


def _is_causal(mask: np.ndarray) -> bool:
    if mask.shape != (B, N, N):
        return False
    tril = np.tril(np.ones((N, N), bool))
    return bool(all(np.array_equal(mask[i], tril) for i in range(mask.shape[0])))


def _numpy_fallback(x, attention_mask, w_qkv, b_qkv, w_proj, b_proj):
    b, n, c = x.shape
    qkv = x @ w_qkv.T + b_qkv
    qkv = qkv.reshape(b, n, 3, H, D).transpose(2, 0, 3, 1, 4)
    q, k, v = qkv[0], qkv[1], qkv[2]
    dots = np.einsum("bhid,bhjd->bhij", q, k) * SCALE
    mask_value = -np.finfo(dots.dtype).max
    dots = np.where(attention_mask[:, None, :, :], dots, mask_value)
    dots = dots - dots.max(axis=-1, keepdims=True)
    e = np.exp(dots)
    attn = e / e.sum(axis=-1, keepdims=True)
    out = np.einsum("bhij,bhjd->bhid", attn, v)
    out = out.reshape(b, n, c)
    return (out @ w_proj.T + b_proj).astype(np.float32)


def kernel(**inputs) -> np.ndarray:
    x = np.asarray(inputs["x"], np.float32)
    mask = np.asarray(inputs["attention_mask"])
    w_qkv = np.asarray(inputs["w_qkv"], np.float32)
    b_qkv = np.asarray(inputs["b_qkv"], np.float32)
    w_proj = np.asarray(inputs["w_proj"], np.float32)
    b_proj = np.asarray(inputs["b_proj"], np.float32)

    if not _is_causal(mask):
        return _numpy_fallback(x, mask, w_qkv, b_qkv, w_proj, b_proj)

    nc = build_bass()
    in_maps = [_core_inputs(c, x, w_qkv, b_qkv, w_proj, b_proj)
               for c in range(8)]
    res = run_bass_kernel_spmd(nc, in_maps, core_ids=list(range(8)))
    out = np.empty((B, N, C), np.float32)
    for c in range(8):
        b = c // 4
        h0 = 4 * (c % 4)
        out[b, P * h0:P * h0 + 512, :] = res.results[c]["out"]
    return out
